# revision 19
# baseline (speedup 1.0000x reference)
"""Trainium2 Bass kernel for fused multi-head causal attention.

Module: out = o_proj(causal_attention(rope_swapped(qkv_proj(x)))).
Shapes: x [2, 2048, 2048], 16 heads, head_dim 128.

Sharding (8 cores): batch (2) x head-group (4 groups of 4 heads).
Each core computes qkv projection + attention for its 4 heads of its
batch, then a partial o_proj against its slice of w_o rows.  The
all-reduce after o_proj is done host-side by summing the 4 partials
per batch (mathematically identical, avoids device collectives).

Device-kernel design notes:
 - QKV and o_proj matmuls run in fp8-e4m3 DoubleRow perf mode (0.5
   cycles/row on PE) with hi+lo error compensation: a ~= hi(a) + lo(a)
   with both parts fp8; a@b ~= ah@bh + al@bh + ah@bl (lo x lo dropped,
   ~0.1% error).  3 chains x half-rate = 0.75x the bf16 PE cost.
   Weights are scaled by 64 host-side so their fp8 parts stay normal;
   the 1/64 is folded into the rope tables / V copy / output copy.
 - DoubleRow contracts 2 k-chunks of 128 per instruction; operands are
   laid out [128 partitions, 2, free] with the paired k-chunks adjacent
   in the free dim (prepared host-side).
 - QKV projection computes Q^T/K^T/V^T ([head_dim, S] layout) directly:
   out = W_slice.T @ x^T, so attention's QK^T matmul needs no transposes.
 - Q/K head pairs are interleaved ([q_ha_lo | q_hb_lo] on 128 partitions)
   so RoPE's rotate_half partner lives at the SAME partition of a sibling
   tile -> full-width DVE ops (3 ops/element, no cross-partition shuffle).
 - Attention is computed in [sq, sk] score orientation, one unit per
   (head, 128-row sq tile): logits = Q_tile^T.T @ K^T streamed in 512-col
   pieces; the causal triangle of the diagonal block is accumulated on
   the PE (ident^T @ cmask); exp runs on the scalar engine with
   accum_out producing the softmax denominator per sq row FOR FREE (no
   PE ones-matmuls); p is normalized in-place on the DVE with the
   per-partition reciprocal, then DMA-transposed (SWDGE via the gpsimd
   queue, off the PE) into [sk, sq] blocks consumed by the PV matmul.
   PV emission lags QK emission by a few units so the exp/normalize/
   transpose latency hides under other units' QK streams; units run in
   DESCENDING sq order so the pipeline fills on big units and drains on
   tiny ones.
 - Causal skip: unit t only processes sk blocks 0..t (62.5% of full).
 - Attention core (QK^T, exp, PV) stays bf16: a single uncompensated
   fp8 tensor there would inject ~3.6% output error.
 - V^T -> V transposes go through the DMA xbar (dma_start_transpose),
   off the PE/psum entirely; V projection runs as 2-acc passes so psum
   banks drain while the next pass computes.
 - Pair-1 qkv weights and the o_proj weights stream in small chunks
   interleaved with the attention unit loop so the (serialized) DMA
   engine never head-of-line blocks the p-transposes.
 - Output rows are assembled [128, 2048] f16 in SBUF and written with
   one wide DMA per s-tile (halves output bytes, 4x fewer DMAs);
   partials are summed in fp32 host-side.
"""

import math

import ml_dtypes
import numpy as np

S = 2048
D = 2048
HD = 128
NH = 16
N_CORES = 8
SQ = 512          # free-dim chunk for matmuls / psum tiles
NJ = S // SQ      # 4 s-chunks
KP = D // 256     # 8 contraction k-pair chunks (DoubleRow: 256 each)
NST = S // 128    # 16 s-tiles of 128
LAG = 4           # PV emission lag (units) behind QK emission
BF16 = ml_dtypes.bfloat16
F16 = np.float16
F8 = ml_dtypes.float8_e4m3

_MODULE_CACHE = {}


def _build_module(with_bias=True):
    from contextlib import ExitStack

    import concourse.bass as bass
    import concourse.bacc as bacc
    import concourse.mybir as mybir
    import concourse.tile as tile

    f32 = mybir.dt.float32
    f16 = mybir.dt.float16
    bf16 = mybir.dt.bfloat16
    fp8 = mybir.dt.float8e4
    DR = mybir.MatmulPerfMode.DoubleRow
    ts = bass.ts

    nc = bacc.Bacc("TRN2", target_bir_lowering=False, debug=False,
                   num_devices=N_CORES)

    # DRAM I/O (identical program on all cores; per-core data differs)
    xh_d = nc.dram_tensor("xh", [1024, 2 * S], fp8, kind="ExternalInput").ap()
    xl_d = nc.dram_tensor("xl", [1024, 2 * S], fp8, kind="ExternalInput").ap()
    wh_d = nc.dram_tensor("wh", [1024, 2, 2, 768], fp8,
                          kind="ExternalInput").ap()
    wl_d = nc.dram_tensor("wl", [1024, 2, 2, 768], fp8,
                          kind="ExternalInput").ap()
    bias_d = (nc.dram_tensor("bias", [12, 256], fp8,
                             kind="ExternalInput").ap() if with_bias else None)
    woh_d = nc.dram_tensor("woh", [256, 2 * D], fp8, kind="ExternalInput").ap()
    wol_d = nc.dram_tensor("wol", [256, 2 * D], fp8, kind="ExternalInput").ap()
    stab_d = nc.dram_tensor("stab", [128, S], bf16, kind="ExternalInput").ap()
    ctab_d = nc.dram_tensor("ctab", [128, S], bf16, kind="ExternalInput").ap()
    cmask_d = nc.dram_tensor("cmask", [128, 128], bf16, kind="ExternalInput").ap()
    ident_d = nc.dram_tensor("ident", [128, 128], bf16, kind="ExternalInput").ap()
    out_d = nc.dram_tensor("out", [S, D], f16, kind="ExternalOutput").ap()

    inv_sqrt_hd = 1.0 / math.sqrt(HD)

    with tile.TileContext(nc) as tc, ExitStack() as ctx:
        # Long-lived pools first; x/w/tab/vt live in an inner scope that is
        # closed after the last QKV pass so the wo/out pools can reuse the
        # space (SBUF pool allocation is a LIFO stack).
        ps = ctx.enter_context(
            tc.tile_pool(name="ps", bufs=7, space=bass.MemorySpace.PSUM))
        apc_ps = ctx.enter_context(
            tc.tile_pool(name="apc", bufs=1, space=bass.MemorySpace.PSUM))
        const_p = ctx.enter_context(tc.tile_pool(name="const", bufs=1))
        qk_p = ctx.enter_context(tc.tile_pool(name="qk", bufs=8))
        v_p = ctx.enter_context(tc.tile_pool(name="v", bufs=4))
        scr_p = ctx.enter_context(tc.tile_pool(name="scr", bufs=6))
        p_p = ctx.enter_context(tc.tile_pool(name="pp", bufs=2))
        pT_p = ctx.enter_context(tc.tile_pool(name="pT", bufs=6))
        dn_p = ctx.enter_context(tc.tile_pool(name="dn", bufs=4))
        af_p = ctx.enter_context(tc.tile_pool(name="af", bufs=1))
        attn_p = ctx.enter_context(tc.tile_pool(name="attn", bufs=4))
        ctx2 = ExitStack()
        x_p = ctx2.enter_context(tc.tile_pool(name="xp", bufs=16))
        w_p = ctx2.enter_context(tc.tile_pool(name="wp", bufs=16))
        tab_p = ctx2.enter_context(tc.tile_pool(name="tab", bufs=2))
        vt_p = ctx2.enter_context(tc.tile_pool(name="vt", bufs=2))

        # x^T (hi+lo fp8, k-pair layout) resident in SBUF; pair-0 weights
        # interleaved so the first QKV pass starts immediately and the
        # stream stays just ahead of PE consumption.
        # the first QK pass reads only w's m-slices 0..3 (cols 0:512 per
        # slot): the V columns (512:768) are deferred out of the critical
        # pass-0 stream and land during the QK passes
        xh_t, xl_t = [], []
        wh0, wl0 = [], []
        for kp in range(KP):
            t = x_p.tile([128, 2, S], fp8, tag="x", name="xh")
            nc.sync.dma_start(out=t[:], in_=xh_d[kp * 128:(kp + 1) * 128, :])
            xh_t.append(t)
            t = w_p.tile([128, 2, 768], fp8, tag="w", name="wh0")
            nc.sync.dma_start(out=t[:, :, 0:512],
                              in_=wh_d[kp * 128:(kp + 1) * 128, 0, :, 0:512])
            wh0.append(t)
            t = x_p.tile([128, 2, S], fp8, tag="x", name="xl")
            nc.sync.dma_start(out=t[:], in_=xl_d[kp * 128:(kp + 1) * 128, :])
            xl_t.append(t)
            t = w_p.tile([128, 2, 768], fp8, tag="w", name="wl0")
            nc.sync.dma_start(out=t[:, :, 0:512],
                              in_=wl_d[kp * 128:(kp + 1) * 128, 0, :, 0:512])
            wl0.append(t)
        # qkv bias first (pass jg=0's stop matmuls need it earliest),
        # then rope tables, then the deferred w V-columns (used from the
        # V passes at ~45us), then attention constants
        bias_t = []
        if with_bias:
            for i in range(12):
                t = const_p.tile([1, 2, 128], fp8, tag=f"b{i}")
                nc.sync.dma_start(out=t[:], in_=bias_d[i:i + 1, :])
                bias_t.append(t)
        stab = tab_p.tile([128, S], bf16, tag="tab")
        ctab = tab_p.tile([128, S], bf16, tag="tab")
        # first-half columns land first: rope j=0,1 unblocks ~1.6us earlier
        nc.sync.dma_start(out=stab[:, 0:1024], in_=stab_d[:, 0:1024])
        nc.sync.dma_start(out=ctab[:, 0:1024], in_=ctab_d[:, 0:1024])
        nc.sync.dma_start(out=stab[:, 1024:S], in_=stab_d[:, 1024:S])
        nc.sync.dma_start(out=ctab[:, 1024:S], in_=ctab_d[:, 1024:S])
        for kp in range(KP):
            nc.sync.dma_start(out=wh0[kp][:, :, 512:768],
                              in_=wh_d[kp * 128:(kp + 1) * 128, 0, :,
                                       512:768])
            nc.sync.dma_start(out=wl0[kp][:, :, 512:768],
                              in_=wl_d[kp * 128:(kp + 1) * 128, 0, :,
                                       512:768])
        cmask = const_p.tile([128, 128], bf16, tag="c0")
        nc.sync.dma_start(out=cmask[:], in_=cmask_d[:])
        ident = const_p.tile([128, 128], bf16, tag="c1")
        nc.sync.dma_start(out=ident[:], in_=ident_d[:])
        ones2 = None
        if with_bias:
            ones2 = const_p.tile([1, 2, SQ], fp8, tag="c4")
            nc.vector.memset(ones2[:], 1.0)

        def pair_w_loaders(pair):
            """Per-kp closures that DMA pair-1 weights; QK columns first,
            V columns behind them (need-order).  Returned tiles are filled
            as the closures run (interleaved into the attention loop)."""
            whs = [w_p.tile([128, 2, 768], fp8, tag="w", name="wh1")
                   for _ in range(KP)]
            wls = [w_p.tile([128, 2, 768], fp8, tag="w", name="wl1")
                   for _ in range(KP)]
            loaders = []
            for kp in range(KP):
                def qk_load(kp=kp):
                    # SP queue: idle during attention (p transposes live on
                    # the Activation queue; SWDGE would clog the Pool engine
                    # that the a2 fp8 conversions need)
                    nc.sync.dma_start(
                        out=whs[kp][:, :, 0:512],
                        in_=wh_d[kp * 128:(kp + 1) * 128, pair, :, 0:512])
                    nc.gpsimd.dma_start(
                        out=wls[kp][:, :, 0:512],
                        in_=wl_d[kp * 128:(kp + 1) * 128, pair, :, 0:512])
                loaders.append(qk_load)
            for kp in range(KP):
                def v_load(kp=kp):
                    nc.sync.dma_start(
                        out=whs[kp][:, :, 512:768],
                        in_=wh_d[kp * 128:(kp + 1) * 128, pair, :, 512:768])
                    nc.sync.dma_start(
                        out=wls[kp][:, :, 512:768],
                        in_=wl_d[kp * 128:(kp + 1) * 128, pair, :, 512:768])
                loaders.append(v_load)
            return (whs, wls), loaders

        def qkv_pass(w_tiles, pair, jms, hooks=None):
            """Accumulate 64*(x @ W + b) for the given (j, mat) pairs via
            3 fp8 DoubleRow chains.  Returns {(j, m): psum AP [128, SQ]}.
            hooks: closures popped one per kp iteration (emission
            interleave for cross-phase overlap)."""
            whs, wls = w_tiles
            accs = {}
            for jm in jms:
                accs[jm] = ps.tile([128, SQ], f32, tag="ps",
                                   name="qkv_acc")[:]
            for kp in range(KP):
                for ci, (cx, cw) in enumerate(
                        [(xh_t, whs), (xl_t, whs), (xh_t, wls)]):
                    for (j, m) in jms:
                        nc.tensor.matmul(
                            accs[(j, m)],
                            cw[kp][:, :, ts(m, 128)],
                            cx[kp][:, :, ts(j, SQ)],
                            start=(kp == 0 and ci == 0),
                            stop=(not with_bias and kp == KP - 1
                                  and ci == 2),
                            perf_mode=DR)
                if hooks:
                    hooks.pop(0)()
            if with_bias:
                for (j, m) in jms:
                    nc.tensor.matmul(
                        accs[(j, m)],
                        bias_t[pair * 6 + m][:, :, :],
                        ones2[:, :, :],
                        start=False, stop=True, perf_mode=DR)
            return accs

        def rope(j, A, B, dsts):
            """A=[lo ha|lo hb], B=[hi ha|hi hb] pair-interleaved psum tiles
            (64x scale); writes per-head contiguous rotated [128, SQ] slices
            into dsts[0] (head a) and dsts[1] (head b).  Tables carry the
            1/64:  rot_lo = lo*sin - hi*cos ; rot_hi = hi*sin + lo*cos."""
            sl = stab[:, ts(j, SQ)]
            cl = ctab[:, ts(j, SQ)]
            # drain psum -> bf16 SBUF on the scalar engine first: frees the
            # psum bank after 2 ops (not 4) and makes every DVE op below
            # all-bf16/SBUF -> 2x DVE rate
            A2 = scr_p.tile([128, SQ], bf16, tag="scr")
            nc.scalar.copy(A2[:], A)
            B2 = scr_p.tile([128, SQ], bf16, tag="scr")
            nc.scalar.copy(B2[:], B)
            t1 = scr_p.tile([128, SQ], bf16, tag="scr")
            nc.vector.tensor_mul(t1[:], A2[:], sl)
            t2 = scr_p.tile([128, SQ], bf16, tag="scr")
            nc.vector.tensor_mul(t2[:], B2[:], cl)
            t3 = scr_p.tile([128, SQ], bf16, tag="scr")
            nc.vector.tensor_mul(t3[:], B2[:], sl)
            t4 = scr_p.tile([128, SQ], bf16, tag="scr")
            nc.vector.tensor_mul(t4[:], A2[:], cl)
            for hh in range(2):
                hs = slice(64 * hh, 64 * hh + 64)
                nc.vector.tensor_sub(dsts[hh][0:64, ts(j, SQ)],
                                     t1[hs, :], t2[hs, :])
                nc.vector.tensor_add(dsts[hh][64:128, ts(j, SQ)],
                                     t3[hs, :], t4[hs, :])

        # attn pair tiles for o_proj fp8 chains: [128, 2, S], slot = head
        a2h = [attn_p.tile([128, 2, S], fp8, tag="attn", name="a2h")
               for _ in range(2)]
        a2l = [attn_p.tile([128, 2, S], fp8, tag="attn", name="a2l")
               for _ in range(2)]

        Exp = mybir.ActivationFunctionType.Exp

        def make_attn(pair, qT, kT, vs):
            """Unit-pipelined [sq, sk] attention for one head pair.
            Returns (emit_qk, emit_pv) where emit_qk(u) emits the QK/exp/
            normalize/transpose stream for unit u and emit_pv(u) the PV
            accumulation (+ per-chunk fp8 convert).  Units are (hh, t)
            with t the 128-row sq tile, DESCENDING t order."""
            units = [(hh, t) for hh in range(2)
                     for t in range(NST - 1, -1, -1)]
            pTs = {}
            apch = {}       # (hh, j) -> psum AP, allocated on first PV
            pv_done = {}    # (hh, j) -> count of PV'd subtiles

            def emit_qk(u):
                hh, t = units[u]
                nblk = t + 1
                npc = (nblk + 3) // 4
                p = p_p.tile([128, S], bf16, tag="p")
                pT = pT_p.tile([128, NST, 128], bf16, tag="pT")
                dacc = dn_p.tile([128, 8], f32, tag="dn")
                lhs = qT[hh][:, 128 * t:128 * (t + 1)]
                for pc in range(npc):
                    w = min(SQ, 128 * nblk - SQ * pc)
                    lg = ps.tile([128, SQ], f32, tag="ps", name="lg")
                    last = (pc == npc - 1)
                    # logits[sq, sk] = Q_t^T.T @ K^T; on the last piece the
                    # causal -9e15 triangle of the diagonal block lands on
                    # the PE itself (ident.T @ cmask) - no DVE hop
                    nc.tensor.matmul(lg[:, 0:w], lhs,
                                     kT[hh][:, SQ * pc:SQ * pc + w],
                                     start=True, stop=not last)
                    if last:
                        nc.tensor.matmul(lg[:, w - 128:w], ident, cmask,
                                         start=False, stop=True)
                    # exp + free softmax denominator (per-sq-row sum)
                    nc.scalar.activation(p[:, SQ * pc:SQ * pc + w],
                                         lg[:, 0:w], Exp,
                                         scale=inv_sqrt_hd,
                                         accum_out=dacc[:, pc:pc + 1])
                rc = dacc[:, 5:6]
                if npc == 1:
                    nc.vector.reciprocal(rc, dacc[:, 0:1])
                else:
                    acc = dacc[:, 4:5]
                    nc.vector.tensor_add(acc, dacc[:, 0:1], dacc[:, 1:2])
                    for pc in range(2, npc):
                        nc.vector.tensor_add(acc, acc, dacc[:, pc:pc + 1])
                    nc.vector.reciprocal(rc, acc)
                # one wide normalize + one wide transpose per unit; SP
                # queue: a DMA issue on the Activation queue would steal
                # ~670ns of exp-dispatch time per transpose
                wt = 128 * nblk
                nc.vector.tensor_scalar_mul(p[:, 0:wt], p[:, 0:wt], rc)
                nc.sync.dma_start_transpose(pT[:, 0:nblk, :], p[:, 0:wt])
                pTs[u] = pT

            def emit_pv(u):
                hh, t = units[u]
                nblk = t + 1
                j, st4 = t // 4, t % 4
                if (hh, j) not in apch:
                    apch[(hh, j)] = apc_ps.tile([128, SQ], f32, tag="apc",
                                                name="apch")[:]
                    pv_done[(hh, j)] = 0
                acc = apch[(hh, j)]
                pT = pTs.pop(u)
                for b in range(nblk):
                    nc.tensor.matmul(acc[:, 128 * st4:128 * st4 + 128],
                                     vs[hh][:, b, :], pT[:, b, :],
                                     start=(b == 0), stop=(b == nblk - 1))
                pv_done[(hh, j)] += 1
                if pv_done[(hh, j)] == 4:
                    # chunk complete: normalized attn^T -> f16, then fp8
                    # hi/lo pair layout for the o_proj chains
                    aft = af_p.tile([128, SQ], f16, tag="af")
                    # DVE copy: keeps the in-order Activation stream free
                    # for exp dispatch
                    nc.vector.tensor_copy(aft[:], acc)
                    nc.gpsimd.tensor_copy(a2h[pair][:, hh, ts(j, SQ)],
                                          aft[:])
                    nc.gpsimd.tensor_sub(a2l[pair][:, hh, ts(j, SQ)],
                                         aft[:],
                                         a2h[pair][:, hh, ts(j, SQ)])
                    del apch[(hh, j)]
            return units, emit_qk, emit_pv

        ncopy = [0]
        wo2 = []

        def oproj_block(st):
            """o_proj partial for s-tile st: out[s,:] = sum_h attn_h @ wo_h
            (fp8 chains over both head pairs); one wide row DMA per tile
            (narrow DMAs for the last tile - shorter shutdown chain)."""
            narrow = (st == 3)   # last block in emission order
            orow = out_p.tile([128, D], f16, tag="outp")
            for eg in range(2):
                ops = [ps.tile([128, SQ], f32, tag="ps", name="oproj")[:]
                       for _ in range(2)]
                for g in range(2):
                    woh_t, wol_t = wo2[g]
                    for ci, (ca, cw) in enumerate(
                            [(a2h[g], woh_t), (a2l[g], woh_t),
                             (a2h[g], wol_t)]):
                        for ei in range(2):
                            e = 2 * eg + ei
                            nc.tensor.matmul(
                                ops[ei],
                                ca[:, :, ts(st, 128)],
                                cw[:, :, ts(e, SQ)],
                                start=(g == 0 and ci == 0),
                                stop=(g == 1 and ci == 2),
                                perf_mode=DR)
                for ei in range(2):
                    e = 2 * eg + ei
                    if ncopy[0] % 2 == 0:
                        nc.scalar.mul(orow[:, ts(e, SQ)], ops[ei],
                                      1.0 / 64.0)
                    else:
                        nc.vector.tensor_scalar_mul(orow[:, ts(e, SQ)],
                                                    ops[ei], 1.0 / 64.0)
                    ncopy[0] += 1
                    if narrow:
                        nc.sync.dma_start(
                            out=out_d[st * 128:(st + 1) * 128,
                                      e * SQ:(e + 1) * SQ],
                            in_=orow[:, ts(e, SQ)])
            if not narrow:
                nc.sync.dma_start(out=out_d[st * 128:(st + 1) * 128, :],
                                  in_=orow[:])

        RUNAHEAD = 6   # unit QKs pre-emitted at prior-phase V-pass
                       # boundaries (bounded by the pT pool depth)

        def attn_steps(emitters, extra=(), blocks_after=None):
            """Emission-step closures for one pair's attention: QK steps
            (starting after the RUNAHEAD prefix) with PV steps trailing;
            `extra` closures (weight DMAs) woven in early; `blocks_after`
            maps a PV index -> o_proj s-tiles computable once that PV's
            chunk is in."""
            units, emit_qk, emit_pv = emitters
            steps = []
            extra = list(extra)
            due = []
            qk_i, pv_i = RUNAHEAD, 0
            while pv_i < len(units):
                if qk_i < len(units):
                    steps.append(lambda u=qk_i: emit_qk(u))
                    qk_i += 1
                    if extra:
                        steps.append(extra.pop(0))
                if qk_i - pv_i > LAG or qk_i == len(units):
                    steps.append(lambda u=pv_i: emit_pv(u))
                    if blocks_after:
                        due.extend(blocks_after.get(pv_i, ()))
                    pv_i += 1
                if due:
                    steps.append(lambda st=due.pop(0): oproj_block(st))
            steps.extend(extra)
            for st in due:
                steps.append(lambda st=st: oproj_block(st))
            return steps

        # ---- Phase A: pair-0 QKV/V; pair-1 weights stream via kp hooks;
        # pair-0's first attention units start at V-pass boundaries so
        # the (Activation-bound) exp pipeline is already hot when phase B
        # starts ----
        qT0 = [qk_p.tile([128, S], bf16, tag="qk", name="qT0")
               for _ in range(2)]
        kT0 = [qk_p.tile([128, S], bf16, tag="qk", name="kT0")
               for _ in range(2)]
        w1_tiles, schedA = pair_w_loaders(1)
        for j in range(NJ):
            accs = qkv_pass((wh0, wl0), 0, [(j, m) for m in range(4)],
                            hooks=schedA)
            rope(j, accs[(j, 0)], accs[(j, 1)], qT0)
            rope(j, accs[(j, 2)], accs[(j, 3)], kT0)
        vts0 = [vt_p.tile([128, S], bf16, tag="vt", name="vt0"),
                vt_p.tile([128, S], bf16, tag="vt", name="vt1")]
        vs0 = [v_p.tile([128, NST, 128], bf16, tag="v", name="v0"),
               v_p.tile([128, NST, 128], bf16, tag="v", name="v1")]
        em0 = make_attn(0, qT0, kT0, vs0)
        ra0 = 0
        for j in range(NJ):
            vacc = qkv_pass((wh0, wl0), 0, [(j, 4), (j, 5)], hooks=schedA)
            for hh in range(2):
                if hh == 0:
                    nc.scalar.mul(vts0[hh][:, ts(j, SQ)],
                                  vacc[(j, 4 + hh)], 1.0 / 64.0)
                else:
                    nc.vector.tensor_scalar_mul(
                        vts0[hh][:, ts(j, SQ)],
                        vacc[(j, 4 + hh)], 1.0 / 64.0)
                nc.sync.dma_start_transpose(
                    vs0[hh][:, 4 * j:4 * j + 4, :],
                    vts0[hh][:, ts(j, SQ)])
            if j >= 1:
                for _ in range(2):
                    em0[1](ra0)
                    ra0 += 1
        while schedA:
            schedA.pop(0)()

        # ---- Phase B: pair-1 QKV/V passes with pair-0 attention steps
        # interleaved one per kp slot (the attention pipeline is
        # Activation-bound; the QKV PE stream hides it); pair-1's first
        # units start at this phase's V-pass boundaries ----
        qT1 = [qk_p.tile([128, S], bf16, tag="qk", name="qT1")
               for _ in range(2)]
        kT1 = [qk_p.tile([128, S], bf16, tag="qk", name="kT1")
               for _ in range(2)]
        schedB = attn_steps(em0)
        for j in range(NJ):
            accs = qkv_pass(w1_tiles, 1, [(j, m) for m in range(4)],
                            hooks=schedB)
            rope(j, accs[(j, 0)], accs[(j, 1)], qT1)
            rope(j, accs[(j, 2)], accs[(j, 3)], kT1)
        vts1 = [vt_p.tile([128, S], bf16, tag="vt", name="vt0"),
                vt_p.tile([128, S], bf16, tag="vt", name="vt1")]
        vs1 = [v_p.tile([128, NST, 128], bf16, tag="v", name="v0"),
               v_p.tile([128, NST, 128], bf16, tag="v", name="v1")]
        em1 = make_attn(1, qT1, kT1, vs1)
        ra1 = 0
        for j in range(NJ):
            vacc = qkv_pass(w1_tiles, 1, [(j, 4), (j, 5)], hooks=schedB)
            for hh in range(2):
                if hh == 0:
                    nc.scalar.mul(vts1[hh][:, ts(j, SQ)],
                                  vacc[(j, 4 + hh)], 1.0 / 64.0)
                else:
                    nc.vector.tensor_scalar_mul(
                        vts1[hh][:, ts(j, SQ)],
                        vacc[(j, 4 + hh)], 1.0 / 64.0)
                nc.sync.dma_start_transpose(
                    vs1[hh][:, 4 * j:4 * j + 4, :],
                    vts1[hh][:, ts(j, SQ)])
            if j >= 1:
                for _ in range(2):
                    if schedB:
                        schedB.pop(0)()
                    em1[1](ra1)
                    ra1 += 1
        while schedB:
            schedB.pop(0)()

        # ---- Phase C: pair-1 attention with o_proj blocks woven in as
        # both pairs' chunks complete (chunk (hh,j) final after PV index
        # 16*hh + 15-4j; blocks 4j..4j+3 need hh=1's) ----
        ctx2.close()   # free x/w/tab/vt SBUF for the wo/out pools
        wo_p = ctx.enter_context(tc.tile_pool(name="wo", bufs=4))
        out_p = ctx.enter_context(tc.tile_pool(name="outp", bufs=3))
        wo_steps = []
        for g in range(2):
            def wo_load(g=g):
                th = wo_p.tile([128, 2, D], fp8, tag="wo", name="woh_t")
                nc.sync.dma_start(
                    out=th[:], in_=woh_d[g * 128:(g + 1) * 128, :])
                tl = wo_p.tile([128, 2, D], fp8, tag="wo", name="wol_t")
                nc.sync.dma_start(
                    out=tl[:], in_=wol_d[g * 128:(g + 1) * 128, :])
                wo2.append((th, tl))
            wo_steps.append(wo_load)
        blocks_after = {19: [12, 13, 14, 15], 23: [8, 9, 10, 11],
                        27: [4, 5, 6, 7], 31: [0, 1, 2, 3]}
        for step in attn_steps(em1, extra=wo_steps,
                               blocks_after=blocks_after):
            step()

    nc.compile()
    return nc


def _split8(a):
    hi = np.clip(a, -240.0, 240.0).astype(F8)
    lo = (a - hi.astype(np.float32)).astype(F8)
    return hi, lo


def _pairify(a):
    """[K, C] -> [K/2, 2C]: row kp*128+p holds k-chunks (2kp, 2kp+1) side
    by side (DoubleRow k-pair layout)."""
    Kd, C = a.shape
    return np.ascontiguousarray(
        a.reshape(Kd // 256, 2, 128, C).transpose(0, 2, 1, 3)
        .reshape(Kd // 2, 2 * C))


def _host_inputs(x, w_qkv, b_qkv, w_o):
    """Build the 8 per-core input maps."""
    x = np.asarray(x, dtype=np.float32)
    w_qkv = np.asarray(w_qkv, dtype=np.float32)
    b_qkv = np.asarray(b_qkv, dtype=np.float32)
    w_o = np.asarray(w_o, dtype=np.float32)

    # rope tables (reference swaps sin/cos roles; we follow the math:
    # q_rot = q*sin(emb) + rotate_half(q)*cos(emb)); 1/64 de-scales the
    # 64x weight scaling used to keep fp8 weight parts in normal range
    inv_freq = 1.0 / (10000.0 ** (np.arange(0, HD, 2, dtype=np.float32) / HD))
    t = np.arange(S, dtype=np.float32)
    freq = np.einsum("s,f->sf", t, inv_freq)          # [S, 64]
    sinT = np.sin(freq).T.astype(np.float32) / 64.0   # [64, S]
    cosT = np.cos(freq).T.astype(np.float32) / 64.0
    stab = np.concatenate([sinT, sinT], 0).astype(BF16)   # [128, S]
    ctab = np.concatenate([cosT, cosT], 0).astype(BF16)

    p_idx = np.arange(128)[:, None]
    f_idx = np.arange(128)[None, :]
    # accumulated as ident.T @ cmask into the [sq, sk] logits:
    # additive mask[sq, sk] = cmask[sq, sk] = -9e15 where sq < sk
    cmask = np.where(p_idx >= f_idx, 0.0, -9e15).astype(BF16)
    ident = np.eye(128, dtype=np.float32).astype(BF16)

    def head_w(h):
        base = h * 3 * HD
        return (w_qkv[:, base:base + HD],
                w_qkv[:, base + HD:base + 2 * HD],
                w_qkv[:, base + 2 * HD:base + 3 * HD])

    def head_b(h):
        base = h * 3 * HD
        return (b_qkv[base:base + HD],
                b_qkv[base + HD:base + 2 * HD],
                b_qkv[base + 2 * HD:base + 3 * HD])

    in_maps = []
    for c in range(N_CORES):
        b = c // 4
        heads = [4 * (c % 4) + i for i in range(4)]
        xT = np.ascontiguousarray(x[b].T)               # [D, S] f32
        xh, xl = _split8(xT)
        xh = _pairify(xh)
        xl = _pairify(xl)

        mats, bvec = [], []
        for pair in range(2):
            ha, hb = heads[2 * pair], heads[2 * pair + 1]
            wq_a, wk_a, wv_a = head_w(ha)
            wq_b, wk_b, wv_b = head_w(hb)
            bq_a, bk_a, bv_a = head_b(ha)
            bq_b, bk_b, bv_b = head_b(hb)
            mats += [
                np.concatenate([wq_a[:, :64], wq_b[:, :64]], 1),
                np.concatenate([wq_a[:, 64:], wq_b[:, 64:]], 1),
                np.concatenate([wk_a[:, :64], wk_b[:, :64]], 1),
                np.concatenate([wk_a[:, 64:], wk_b[:, 64:]], 1),
                wv_a, wv_b,
            ]
            bvec += [
                np.concatenate([bq_a[:64], bq_b[:64]]),
                np.concatenate([bq_a[64:], bq_b[64:]]),
                np.concatenate([bk_a[:64], bk_b[:64]]),
                np.concatenate([bk_a[64:], bk_b[64:]]),
                bv_a, bv_b,
            ]
        w_all = np.concatenate(mats, 1) * 64.0               # [D, 1536]

        def _w_layout(a):
            # [2048, 1536] -> rows kp*128+p, cols pair*1536 + slot*768 + c
            return np.ascontiguousarray(
                a.reshape(KP, 2, 128, 2, 768).transpose(0, 2, 3, 1, 4)
                .reshape(1024, 3072))

        wh8, wl8 = _split8(w_all)
        wh8 = _w_layout(wh8)
        wl8 = _w_layout(wl8)
        bias_rows = np.zeros((12, 256), dtype=F8)
        for i, bv in enumerate(bvec):
            bh, bl = _split8(bv * 64.0)
            bias_rows[i, :128] = bh
            bias_rows[i, 128:] = bl

        wo_all = np.concatenate(
            [w_o[h * HD:(h + 1) * HD, :] for h in heads], 0) * 64.0  # [512,D]
        woh8, wol8 = _split8(wo_all)
        # [512, D] -> rows g*128+p, cols slot*D+e (slot = head-in-pair)
        woh8 = np.ascontiguousarray(
            woh8.reshape(2, 2, 128, D).transpose(0, 2, 1, 3).reshape(256, 2 * D))
        wol8 = np.ascontiguousarray(
            wol8.reshape(2, 2, 128, D).transpose(0, 2, 1, 3).reshape(256, 2 * D))

        in_maps.append({
            "xh": xh, "xl": xl, "wh": wh8, "wl": wl8, "bias": bias_rows,
            "woh": woh8, "wol": wol8,
            "stab": stab, "ctab": ctab, "cmask": cmask, "ident": ident,
        })
    return in_maps


def _run(in_maps, with_bias, trace=False):
    from concourse.bass_utils import run_bass_kernel_spmd
    key = f"nc{int(with_bias)}"
    if key not in _MODULE_CACHE:
        _MODULE_CACHE[key] = _build_module(with_bias=with_bias)
    nc = _MODULE_CACHE[key]
    _MODULE_CACHE["nc"] = nc   # the module actually executed (for tooling)
    if not with_bias:
        in_maps = [{k: v for k, v in m.items() if k != "bias"}
                   for m in in_maps]
    return run_bass_kernel_spmd(nc, in_maps, core_ids=list(range(N_CORES)),
                                trace=trace)


def kernel(x, w_qkv, b_qkv, w_o, b_o, _trace=False, _return_res=False):
    in_maps = _host_inputs(x, w_qkv, b_qkv, w_o)
    # zero qkv-bias fast path: drops the 48 rank-1 bias matmuls on the PE
    with_bias = bool(np.any(np.asarray(b_qkv)))
    res = _run(in_maps, with_bias, trace=_trace)
    out = np.zeros((2, S, D), dtype=np.float32)
    for c in range(N_CORES):
        out[c // 4] += res.results[c]["out"].astype(np.float32)
    out += np.asarray(b_o, dtype=np.float32)[None, None, :]
    if _return_res:
        return out, res
    return out
